# revision 1
# baseline (speedup 1.0000x reference)
"""Trainium2 Bass kernel for nn_Attention_Critic (gnn_message_passing).

Strategy: data-parallel over the batch (8 cores x 4096), feature-major
layout on chip ([features, batch]), BatchNorm folded into first-layer
weights (stats via one tiny cross-core AllReduce), attention-weight
products folded on host (sel@key^T), attention dots via PE column-reduce
matmuls, softmax computed batch-major over iteration PAIRS, weights
transposed back via the DMA xbar and broadcast via stride-0 DRAM reads.
bf16 matmuls with fp32 PSUM/stats.
"""
import os
import sys

sys.path.insert(0, "/opt/trn_rl_repo")

import numpy as np
import ml_dtypes
from contextlib import ExitStack

import concourse.bass as bass
import concourse.tile as tile
from concourse import bacc, mybir
from concourse.bass_utils import run_bass_kernel_spmd

# Pin every activation to the natural_log_exp_and_others table set (covers
# Exp/Ln/Prelu/Identity/Square/Copy) so the whole kernel needs exactly one
# ACT_TABLE_LOAD instead of thrashing between per-function sets.
_ORIG_GAT = bacc.get_activation_tables


def _pinned_tables(arch):
    t = _ORIG_GAT(arch)
    return {k: (v if k == "natural_log_exp_and_others" else set())
            for k, v in t.items()}


bacc.get_activation_tables = _pinned_tables

NA, B, H = 3, 32768, 128
EPS = 1e-5
NCORES = 8
BS = B // NCORES          # 4096 per core
NT = 512                  # batch tile
ITERS = BS // NT          # 8
NPAIR = ITERS // 2        # 4 iteration pairs
SCALE = 1.0 / np.sqrt(H)

bf16 = mybir.dt.bfloat16
f32 = mybir.dt.float32

BLOCKS = [("en", 0, 6), ("oa0", 7, 4), ("oa1", 12, 4), ("g0", 17, 2),
          ("g1", 20, 2), ("g2", 23, 2), ("senc", 26, 20)]
BLOCK_STAT = {"en": 0, "oa0": 6, "oa1": 10, "g0": 14, "g1": 16, "g2": 18,
              "senc": 0}
BIGW = (["wsk0", "wsk1", "aval0", "aval1", "mcrit", "cvalw"]
        + [f"m_en{n}" for n in range(NA)] + [f"m_ov0{n}" for n in range(NA)]
        + [f"m_ov1{n}" for n in range(NA)] + [f"cw1a{n}" for n in range(NA)]
        + [f"cw1b{n}" for n in range(NA)])
BIASC = ["avb0", "avb1", "mb0", "mb1", "mb2", "cvb", "cb10", "cb11", "cb12"]


def _b16(x):
    return np.asarray(x, np.float32).astype(ml_dtypes.bfloat16)


def _prep_ent_blocks(s, a, lo, hi):
    rows = []
    for n in range(NA):
        sn = s[n, lo:hi].T
        an = a[n, lo:hi].T
        ones = np.ones((1, hi - lo), np.float32)
        rows += [sn[0:4], an[0:2], ones]
        rows += [sn[4:8], ones, sn[8:12], ones]
        rows += [sn[12:14], ones, sn[14:16], ones, sn[16:18], ones]
        rows += [sn[0:4], an[0:2], sn[4:18], ones]
    return np.ascontiguousarray(np.concatenate(rows, 0), dtype=np.float32)


def _prep_l1w(inp):
    out = np.zeros((141, 128), np.float32)
    for n in range(NA):
        o = 47 * n
        out[o + 0:o + 6] = inp["en_W"][n]
        out[o + 6] = inp["en_b"][n]
        out[o + 7:o + 11] = inp["oa_W"][n]
        out[o + 11] = inp["oa_b"][n]
        out[o + 12:o + 16] = inp["oa_W"][n]
        out[o + 16] = inp["oa_b"][n]
        out[o + 17:o + 19] = inp["goal_W"][n]
        out[o + 19] = inp["goal_b"][n]
        out[o + 20:o + 22] = inp["goal_W"][n]
        out[o + 22] = inp["goal_b"][n]
        out[o + 23:o + 25] = inp["goal_W"][n]
        out[o + 25] = inp["goal_b"][n]
        out[o + 26:o + 30] = inp["senc_W"][n][0:4]
        out[o + 32:o + 46] = inp["senc_W"][n][4:18]
        out[o + 46] = inp["senc_b"][n]
    return out


def _prep_bigw(inp):
    w = {}
    w["wsk0"] = inp["asel_W"][0] @ inp["akey_W"][0].T
    w["wsk1"] = inp["asel_W"][1] @ inp["akey_W"][1].T
    w["aval0"] = inp["aval_W"][0]
    w["aval1"] = inp["aval_W"][1]
    w["mcrit"] = inp["ckey_W"][0] @ inp["csel_W"][0].T
    w["cvalw"] = inp["cval_W"][0]
    for n in range(NA):
        w[f"m_en{n}"] = inp["merge_W"][n, 0:128]
        w[f"m_ov0{n}"] = inp["merge_W"][n, 128:256]
        w[f"m_ov1{n}"] = inp["merge_W"][n, 256:384]
        w[f"cw1a{n}"] = inp["cW1"][n, 0:128]
        w[f"cw1b{n}"] = inp["cW1"][n, 128:256]
    return _b16(np.concatenate([w[k] for k in BIGW], 0))


def _prep_bias(inp):
    cols = [inp["aval_b"][0], inp["aval_b"][1],
            inp["merge_b"][0], inp["merge_b"][1], inp["merge_b"][2],
            inp["cval_b"][0], inp["cb1"][0], inp["cb1"][1], inp["cb1"][2]]
    return np.stack(cols, 1).astype(np.float32)


_NC_CACHE = {}


def _build_nc():
    nc = bacc.Bacc("TRN2", target_bir_lowering=False, debug=False,
                   num_devices=NCORES)
    entd = nc.dram_tensor("entd", [141, BS], f32, kind="ExternalInput")
    l1wd = nc.dram_tensor("l1wd", [141, 128], f32, kind="ExternalInput")
    bigwd = nc.dram_tensor("bigwd", [21 * 128, 128], bf16, kind="ExternalInput")
    cw2d = nc.dram_tensor("cw2d", [NA * 128, 2], bf16, kind="ExternalInput")
    biasd = nc.dram_tensor("biasd", [128, 9], f32, kind="ExternalInput")
    cb2d = nc.dram_tensor("cb2d", [2, NA], f32, kind="ExternalInput")
    outd = nc.dram_tensor("outd", [6, BS], f32, kind="ExternalOutput")

    cc_in = nc.dram_tensor("cc_in", [60, 2], f32)
    cc_out = nc.dram_tensor("cc_out", [60, 2], f32, addr_space="Shared")
    wscrd = nc.dram_tensor("wscrd", [NPAIR, 4, 64, 128], bf16)

    with tile.TileContext(nc) as tc, ExitStack() as ctx:
        wp = ctx.enter_context(tc.tile_pool(name="wp", bufs=1))
        io = ctx.enter_context(tc.tile_pool(name="io", bufs=1))
        wk = ctx.enter_context(tc.tile_pool(name="wk", bufs=2))
        pp = ctx.enter_context(tc.tile_pool(name="pp", bufs=1, space="PSUM"))

        big = {}
        for idx, name in enumerate(BIGW):
            t = wp.tile([128, 128], bf16, name=f"bw_{name}")
            nc.sync.dma_start(t[:], bigwd[128 * idx:128 * (idx + 1), :])
            big[name] = t
        cw2 = []
        for n in range(NA):
            t = wp.tile([128, 2], bf16, name=f"cw2_{n}")
            nc.sync.dma_start(t[:], cw2d[128 * n:128 * (n + 1), :])
            cw2.append(t)
        biast = wp.tile([128, 9], f32)
        nc.sync.dma_start(biast[:], biasd[:, :])
        bcol = {name: biast[:, i:i + 1] for i, name in enumerate(BIASC)}
        cb2t = wp.tile([2, NA], f32)
        nc.sync.dma_start(cb2t[:], cb2d[:, :])
        onesb = wp.tile([128, 1], bf16)
        nc.vector.memset(onesb[:], 1.0)
        zbias = wp.tile([128, 1], f32)
        nc.vector.memset(zbias[:], 0.0)

        GRP = {"en": ("A", 0, 6), "oa0": ("A", 32, 4), "oa1": ("A", 64, 4),
               "g0": ("B", 0, 2), "g1": ("B", 32, 2), "g2": ("B", 64, 2),
               "senc": ("C", 0, 20)}
        ebC = {}
        for n in range(NA):
            o = 47 * n
            t = io.tile([21, BS], bf16, name=f"ebC{n}")
            nc.gpsimd.dma_start(t[:], entd[o + 26:o + 47, :])
            ebC[n] = t

        # ---------- stats ----------
        for n in range(NA):
            sq8 = wp.tile([20, 8], f32, name=f"sq8_{n}")
            for c in range(8):
                sqp = pp.tile([20, 512], f32, name="sqp", tag="T3")
                nc.scalar.activation(
                    sqp[:], ebC[n][0:20, 512 * c:512 * (c + 1)],
                    mybir.ActivationFunctionType.Square,
                    accum_out=sq8[:, c:c + 1])
            sumq = wp.tile([20, 1], f32, name=f"sumq_{n}")
            nc.vector.tensor_reduce(out=sumq[:], in_=sq8[:],
                                    op=mybir.AluOpType.add,
                                    axis=mybir.AxisListType.X)
            sumx = wp.tile([20, 1], f32, name=f"sumx_{n}")
            nc.vector.tensor_reduce(out=sumx[:], in_=ebC[n][0:20, :],
                                    op=mybir.AluOpType.add,
                                    axis=mybir.AxisListType.X)
            nc.sync.dma_start(cc_in[20 * n:20 * n + 20, 0:1], sumx[:])
            nc.sync.dma_start(cc_in[20 * n:20 * n + 20, 1:2], sumq[:])
        nc.gpsimd.collective_compute(
            "AllReduce", mybir.AluOpType.add,
            replica_groups=[list(range(NCORES))],
            ins=[cc_in[:, :]], outs=[cc_out[:, :]])
        gst = wp.tile([60, 2], f32)
        nc.sync.dma_start(gst[:], cc_out[:, :])
        mean = wp.tile([60, 1], f32)
        nc.vector.tensor_scalar_mul(mean[:], gst[:, 0:1], 1.0 / B)
        ex2 = wp.tile([60, 1], f32)
        nc.vector.tensor_scalar_mul(ex2[:], gst[:, 1:2], 1.0 / B)
        m2 = wp.tile([60, 1], f32)
        nc.vector.tensor_mul(m2[:], mean[:], mean[:])
        var = wp.tile([60, 1], f32)
        nc.vector.tensor_sub(var[:], ex2[:], m2[:])
        epst = wp.tile([60, 1], f32)
        nc.vector.memset(epst[:], EPS)
        lnv = wp.tile([60, 1], f32)
        nc.scalar.activation(lnv[:], var[:], mybir.ActivationFunctionType.Ln,
                             bias=epst[:])
        std = wp.tile([60, 1], f32)
        nc.scalar.activation(std[:], lnv[:], mybir.ActivationFunctionType.Exp,
                             scale=0.5)
        rstd = wp.tile([60, 1], f32)
        nc.vector.reciprocal(rstd[:], std[:])
        meanb = wp.tile([60, 1], bf16)
        nc.vector.tensor_copy(meanb[:], mean[:])

        # ---------- fold first-layer weights ----------
        GSIZE = {"A": 69, "B": 67, "C": 21}
        lwg, blkg, rsbg, mbbg = {}, {}, {}, {}
        for n in range(NA):
            for gname in "ABC":
                gsz = GSIZE[gname]
                lwg[(n, gname)] = wp.tile([gsz, 128], f32, name=f"lw{n}{gname}")
                blkg[(n, gname)] = wp.tile([gsz, 128], bf16,
                                           name=f"blk{n}{gname}")
                rsbg[(n, gname)] = wp.tile([gsz, 1], f32, name=f"rsb{n}{gname}")
                mbbg[(n, gname)] = wp.tile([gsz, 1], bf16,
                                           name=f"mbb{n}{gname}")
        blk = {}
        for n in range(NA):
            o = 47 * n
            for bname, st, K in BLOCKS:
                gname, base, _ = GRP[bname]
                so = 20 * n + BLOCK_STAT[bname]
                lw = lwg[(n, gname)]
                bw = blkg[(n, gname)]
                rsb = rsbg[(n, gname)]
                mbb = mbbg[(n, gname)]
                nc.scalar.dma_start(lw[base:base + K, :],
                                    l1wd[o + st:o + st + K, :])
                braw = wk.tile([1, 128], f32, name="brawtmp", bufs=3)
                nc.scalar.dma_start(braw[:],
                                    l1wd[o + st + K:o + st + K + 1, :])
                nc.sync.dma_start(rsb[base:base + K, :], rstd[so:so + K, :])
                nc.sync.dma_start(mbb[base:base + K, :], meanb[so:so + K, :])
                nc.vector.tensor_scalar_mul(bw[base:base + K, :],
                                            lw[base:base + K, :],
                                            rsb[base:base + K, :])
                pb = pp.tile([1, 128], f32, name="pbias", tag="T3")
                nc.tensor.matmul(pb[:], mbb[base:base + K, :],
                                 bw[base:base + K, :], start=True, stop=True)
                brow = wk.tile([1, 128], bf16, name="browtmp", bufs=3)
                nc.vector.tensor_sub(brow[:], braw[:], pb[:])
                nc.sync.dma_start(bw[base + K:base + K + 1, :], brow[:])
                blk[(n, bname)] = bw[base:base + K + 1, :]

        # ---------- main loop: iteration PAIRS ----------
        # PSUM tags (8 banks):
        #  T0[2]: l1p0, v0p, kmpa     T1[2]: l1p1, v1pa, cvpa
        #  T2[2]: l1pG, skp, mp2, hp2 T3[1]: l1p3, v1pb, kmpb, cvpb, qp
        #  T5[1]: lgp, clg
        LR = mybir.ActivationFunctionType.Prelu
        for ip in range(NPAIR):
            psl = slice(ip * 2 * NT, (ip + 1) * 2 * NT)
            sa = {}
            se_t = {}
            l1x_t = {}
            ebg = {}
            for n in range(NA):
                o = 47 * n
                ebA = wk.tile([69, 2 * NT], bf16, name=f"ebA{n}", bufs=2)
                ebB = wk.tile([67, 2 * NT], bf16, name=f"ebB{n}", bufs=2)
                for bname, st, K in BLOCKS:
                    g, base, _ = GRP[bname]
                    if g == "C":
                        continue
                    t = ebA if g == "A" else ebB
                    nc.gpsimd.dma_start(t[base:base + K + 1, :],
                                        entd[o + st:o + st + K + 1, psl])
                    ebg[(n, bname)] = t[base:base + K + 1, :]
            for n in range(NA):
                lgp = pp.tile([128, 40], f32, name="lgp", tag="T5")
                vals0_t = wk.tile([128, 2048], bf16, name="vals0", bufs=2)
                vals1_t = wk.tile([128, 3072], bf16, name="vals1", bufs=2)
                for h in range(2):
                    it = 2 * ip + h
                    sl = slice(it * NT, (it + 1) * NT)
                    hsl = slice(h * NT, (h + 1) * NT)
                    l1p0 = pp.tile([128, 1024], f32, name="l1p0", tag="T0")
                    l1p1 = pp.tile([128, 1024], f32, name="l1p1", tag="T1")
                    l1pG = pp.tile([128, 1024], f32, name="l1pG", tag="T2")
                    l1p3 = pp.tile([128, 512], f32, name="l1p3", tag="T3")
                    dests = {"en": (l1p0, 0), "oa0": (l1p0, 512),
                             "oa1": (l1p1, 0), "g0": (l1p1, 512),
                             "g1": (l1pG, 0), "g2": (l1pG, 512),
                             "senc": (l1p3, 0)}
                    for bname, st, K in BLOCKS:
                        pt, off = dests[bname]
                        rhs = (ebC[n][:, sl] if bname == "senc"
                               else ebg[(n, bname)][:, hsl])
                        nc.tensor.matmul(pt[:, off:off + NT],
                                         blk[(n, bname)], rhs,
                                         start=True, stop=True)
                    l1x = wk.tile([128, 3072], bf16, name="l1x", bufs=3)
                    se = wk.tile([128, 512], bf16, name="se", bufs=6)
                    nc.scalar.activation(l1x[:, 0:1024], l1p0[:], LR,
                                         bias=zbias[:], alpha=0.01)
                    nc.scalar.activation(l1x[:, 1024:2048], l1p1[:], LR,
                                         bias=zbias[:], alpha=0.01)
                    nc.scalar.activation(l1x[:, 2048:3072], l1pG[:], LR,
                                         bias=zbias[:], alpha=0.01)
                    nc.scalar.activation(se[:], l1p3[:], LR,
                                         bias=zbias[:], alpha=0.01)
                    l1x_t[(h, n)] = l1x
                    se_t[(h, n)] = se
                    skp = pp.tile([128, 1024], f32, name="skp", tag="T2")
                    en_ = l1x[:, 0:512]
                    nc.tensor.matmul(skp[:, 0:512], big["wsk0"][:], en_,
                                     start=True, stop=True)
                    nc.tensor.matmul(skp[:, 512:1024], big["wsk1"][:], en_,
                                     start=True, stop=True)
                    selk = wk.tile([128, 1024], bf16, name="selk", bufs=4)
                    nc.vector.tensor_copy(selk[:], skp[:])
                    prs = []
                    for p in range(5):
                        sk = selk[:, 0:512] if p < 2 else selk[:, 512:1024]
                        enc = l1x[:, 512 * (p + 1):512 * (p + 2)]
                        pr = wk.tile([128, 512], bf16, name="pr", bufs=4)
                        peng = nc.gpsimd if p >= 3 else nc.vector
                        peng.tensor_tensor(out=pr[:], in0=sk, in1=enc,
                                           op=mybir.AluOpType.mult)
                        prs.append(pr)
                    for p in range(5):
                        for t in range(4):
                            col = 5 * (4 * h + t) + p
                            nc.tensor.matmul(lgp[:, col:col + 1],
                                             prs[p][:, 128 * t:128 * (t + 1)],
                                             onesb[:], start=True, stop=True)
                    v0p = pp.tile([128, 1024], f32, name="v0p", tag="T0")
                    nc.tensor.matmul(v0p[:, 0:512], big["aval0"][:],
                                     l1x[:, 512:1024], start=True, stop=True)
                    nc.tensor.matmul(v0p[:, 512:1024], big["aval0"][:],
                                     l1x[:, 1024:1536], start=True, stop=True)
                    nc.scalar.activation(vals0_t[:, 1024 * h:1024 * (h + 1)],
                                         v0p[:], LR, bias=bcol["avb0"],
                                         alpha=0.01)
                    v1pa = pp.tile([128, 1024], f32, name="v1pa", tag="T1")
                    v1pb = pp.tile([128, 512], f32, name="v1pb", tag="T3")
                    nc.tensor.matmul(v1pa[:, 0:512], big["aval1"][:],
                                     l1x[:, 1536:2048], start=True, stop=True)
                    nc.tensor.matmul(v1pa[:, 512:1024], big["aval1"][:],
                                     l1x[:, 2048:2560], start=True, stop=True)
                    nc.tensor.matmul(v1pb[:], big["aval1"][:],
                                     l1x[:, 2560:3072], start=True, stop=True)
                    nc.scalar.activation(vals1_t[:, 1536 * h:1536 * h + 1024],
                                         v1pa[:], LR, bias=bcol["avb1"],
                                         alpha=0.01)
                    nc.scalar.activation(
                        vals1_t[:, 1536 * h + 1024:1536 * h + 1536],
                        v1pb[:], LR, bias=bcol["avb1"], alpha=0.01)
                ebm = wk.tile([128, 40], bf16, name="ebm")
                nc.scalar.activation(ebm[:], lgp[:],
                                     mybir.ActivationFunctionType.Exp,
                                     scale=SCALE)
                den = wk.tile([128, 16], f32, name="den")
                nc.vector.tensor_reduce(
                    out=den[:].rearrange("p (t g) -> p t g", g=2)[:, :, 0:1],
                    in_=ebm[:].rearrange("p (t c) -> p t c", c=5)[:, :, 0:2],
                    op=mybir.AluOpType.add, axis=mybir.AxisListType.X)
                nc.vector.tensor_reduce(
                    out=den[:].rearrange("p (t g) -> p t g", g=2)[:, :, 1:2],
                    in_=ebm[:].rearrange("p (t c) -> p t c", c=5)[:, :, 2:5],
                    op=mybir.AluOpType.add, axis=mybir.AxisListType.X)
                rec = wk.tile([128, 16], f32, name="rec")
                nc.vector.reciprocal(rec[:], den[:])
                wbm32 = wk.tile([128, 128], bf16, name="wbm32")
                nc.vector.tensor_tensor(
                    out=wbm32[:, 0:64].rearrange("p (t c) -> p t c", c=8)
                    [:, :, 0:2],
                    in0=ebm[:].rearrange("p (t c) -> p t c", c=5)[:, :, 0:2],
                    in1=rec[:].rearrange("p (t g) -> p t g", g=2)[:, :, 0:1]
                    .broadcast_to((128, 8, 2)),
                    op=mybir.AluOpType.mult)
                nc.vector.tensor_tensor(
                    out=wbm32[:, 0:64].rearrange("p (t c) -> p t c", c=8)
                    [:, :, 2:5],
                    in0=ebm[:].rearrange("p (t c) -> p t c", c=5)[:, :, 2:5],
                    in1=rec[:].rearrange("p (t g) -> p t g", g=2)[:, :, 1:2]
                    .broadcast_to((128, 8, 3)),
                    op=mybir.AluOpType.mult)
                wfmT = wk.tile([128, 128], bf16, name="wfmT")
                nc.scalar.dma_start_transpose(wfmT[:], wbm32[:])
                nc.scalar.dma_start(wscrd[ip, n, :, :], wfmT[0:64, :])
                mp2 = pp.tile([128, 1024], f32, name="mp2", tag="T2")
                for h in range(2):
                    nc.tensor.matmul(mp2[:, 512 * h:512 * (h + 1)],
                                     big[f"m_en{n}"][:],
                                     l1x_t[(h, n)][:, 0:512],
                                     start=True, stop=False)
                scs = []
                for p in range(5):
                    wrow = wscrd[ip, n, :, :] \
                        .rearrange("(t c) b -> t c b", c=8)[:, p:p + 1, :] \
                        .rearrange("t a b -> a t b") \
                        .broadcast_to((128, 8, 128))
                    wb_ = wk.tile([128, 1024], bf16, name="wb", bufs=3)
                    _qeng = [nc.sync, nc.gpsimd, nc.sync, nc.gpsimd,
                             nc.sync][p]
                    _qeng.dma_start(
                        wb_[:].rearrange("p (t b) -> p t b", b=128), wrow)
                    sc = wk.tile([128, 1024], bf16, name="sc", bufs=3)
                    if p < 2:
                        vin = vals0_t[:, :].rearrange(
                            "p (h q b) -> p h q b", h=2, q=2)[:, :, p, :]
                    else:
                        vin = vals1_t[:, :].rearrange(
                            "p (h q b) -> p h q b", h=2, q=3)[:, :, p - 2, :]
                    seng = nc.gpsimd if p in (0, 2) else nc.vector
                    seng.tensor_tensor(
                        out=sc[:].rearrange("p (h b) -> p h b", h=2),
                        in0=vin, in1=wb_[:].rearrange("p (h b) -> p h b", h=2),
                        op=mybir.AluOpType.mult)
                    scs.append(sc)
                ov0 = wk.tile([128, 1024], bf16, name="ov0", bufs=2)
                nc.vector.tensor_tensor(out=ov0[:], in0=scs[0][:],
                                        in1=scs[1][:], op=mybir.AluOpType.add)
                ov1t = wk.tile([128, 1024], bf16, name="ov1t", bufs=2)
                nc.gpsimd.tensor_tensor(out=ov1t[:], in0=scs[2][:],
                                        in1=scs[3][:], op=mybir.AluOpType.add)
                ov1 = wk.tile([128, 1024], bf16, name="ov1", bufs=2)
                nc.vector.tensor_tensor(out=ov1[:], in0=ov1t[:],
                                        in1=scs[4][:], op=mybir.AluOpType.add)
                for h in range(2):
                    nc.tensor.matmul(mp2[:, 512 * h:512 * (h + 1)],
                                     big[f"m_ov0{n}"][:],
                                     ov0[:, 512 * h:512 * (h + 1)],
                                     start=False, stop=False)
                    nc.tensor.matmul(mp2[:, 512 * h:512 * (h + 1)],
                                     big[f"m_ov1{n}"][:],
                                     ov1[:, 512 * h:512 * (h + 1)],
                                     start=False, stop=True)
                for h in range(2):
                    sa_n = wk.tile([128, 512], bf16, name="sa", bufs=7)
                    nc.scalar.activation(sa_n[:],
                                         mp2[:, 512 * h:512 * (h + 1)], LR,
                                         bias=bcol[f"mb{n}"], alpha=0.01)
                    sa[(h, n)] = sa_n
            # ---- critic ----
            keysM = wk.tile([128, 3072], bf16, name="keysM", bufs=2)
            cval = wk.tile([128, 3072], bf16, name="cval", bufs=2)
            for h in range(2):
                kmpa = pp.tile([128, 1024], f32, name="kmpa", tag="T0")
                kmpb = pp.tile([128, 512], f32, name="kmpb", tag="T3")
                nc.tensor.matmul(kmpa[:, 0:512], big["mcrit"][:],
                                 sa[(h, 0)][:], start=True, stop=True)
                nc.tensor.matmul(kmpa[:, 512:1024], big["mcrit"][:],
                                 sa[(h, 1)][:], start=True, stop=True)
                nc.tensor.matmul(kmpb[:], big["mcrit"][:], sa[(h, 2)][:],
                                 start=True, stop=True)
                nc.vector.tensor_copy(keysM[:, 1536 * h:1536 * h + 1024],
                                      kmpa[:])
                nc.vector.tensor_copy(
                    keysM[:, 1536 * h + 1024:1536 * h + 1536], kmpb[:])
                cvpa = pp.tile([128, 1024], f32, name="cvpa", tag="T1")
                cvpb = pp.tile([128, 512], f32, name="cvpb", tag="T3")
                nc.tensor.matmul(cvpa[:, 0:512], big["cvalw"][:],
                                 sa[(h, 0)][:], start=True, stop=True)
                nc.tensor.matmul(cvpa[:, 512:1024], big["cvalw"][:],
                                 sa[(h, 1)][:], start=True, stop=True)
                nc.tensor.matmul(cvpb[:], big["cvalw"][:], sa[(h, 2)][:],
                                 start=True, stop=True)
                nc.scalar.activation(cval[:, 1536 * h:1536 * h + 1024],
                                     cvpa[:], LR, bias=bcol["cvb"],
                                     alpha=0.01)
                nc.scalar.activation(
                    cval[:, 1536 * h + 1024:1536 * h + 1536],
                    cvpb[:], LR, bias=bcol["cvb"], alpha=0.01)
            clg = pp.tile([128, 48], f32, name="clg", tag="T5")
            for h in range(2):
                for i in range(NA):
                    js = [j for j in range(NA) if j != i]
                    for k, j in enumerate(js):
                        prc = wk.tile([128, 512], bf16, name="prc", bufs=3)
                        nc.vector.tensor_tensor(
                            out=prc[:], in0=se_t[(h, i)][:],
                            in1=keysM[:, 1536 * h + 512 * j:
                                      1536 * h + 512 * (j + 1)],
                            op=mybir.AluOpType.mult)
                        c = 2 * i + k
                        for t in range(4):
                            col = 6 * (4 * h + t) + c
                            nc.tensor.matmul(
                                clg[:, col:col + 1],
                                prc[:, 128 * t:128 * (t + 1)],
                                onesb[:], start=True, stop=True)
            cebm = wk.tile([128, 48], bf16, name="cebm")
            nc.scalar.activation(cebm[:], clg[:],
                                 mybir.ActivationFunctionType.Exp, scale=SCALE)
            cden = wk.tile([128, 24], f32, name="cden")
            nc.vector.tensor_reduce(
                out=cden[:].rearrange("p (t i) -> p t i", i=3)
                    .rearrange("p t i -> p t i ()"),
                in_=cebm[:].rearrange("p (t i k) -> p t i k", i=3, k=2),
                op=mybir.AluOpType.add, axis=mybir.AxisListType.X)
            crec = wk.tile([128, 24], f32, name="crec")
            nc.vector.reciprocal(crec[:], cden[:])
            cwbm32 = wk.tile([128, 128], bf16, name="cwbm32")
            nc.vector.tensor_tensor(
                out=cwbm32[:, 0:64].rearrange("p (t c) -> p t c", c=8)
                [:, :, 0:6].rearrange("p t (i k) -> p t i k", k=2),
                in0=cebm[:].rearrange("p (t i k) -> p t i k", i=3, k=2),
                in1=crec[:].rearrange("p (t i u) -> p t i u", i=3, u=1)
                .broadcast_to((128, 8, 3, 2)),
                op=mybir.AluOpType.mult)
            cwfmT = wk.tile([128, 128], bf16, name="cwfmT")
            nc.scalar.dma_start_transpose(cwfmT[:], cwbm32[:])
            nc.scalar.dma_start(wscrd[ip, 3, :, :], cwfmT[0:64, :])
            for i in range(NA):
                js = [j for j in range(NA) if j != i]
                hp2 = pp.tile([128, 1024], f32, name="hp2", tag="T2")
                for h in range(2):
                    nc.tensor.matmul(hp2[:, 512 * h:512 * (h + 1)],
                                     big[f"cw1a{i}"][:], se_t[(h, i)][:],
                                     start=True, stop=False)
                cscs = []
                for k, j in enumerate(js):
                    c = 2 * i + k
                    wrow = wscrd[ip, 3, :, :] \
                        .rearrange("(t c) b -> t c b", c=8)[:, c:c + 1, :] \
                        .rearrange("t a b -> a t b") \
                        .broadcast_to((128, 8, 128))
                    cwb = wk.tile([128, 1024], bf16, name="cwb", bufs=3)
                    _qeng = [nc.sync, nc.gpsimd][k]
                    _qeng.dma_start(
                        cwb[:].rearrange("p (t b) -> p t b", b=128), wrow)
                    csc = wk.tile([128, 1024], bf16, name="csc", bufs=3)
                    ceng = nc.gpsimd if k == 0 else nc.vector
                    ceng.tensor_tensor(
                        out=csc[:].rearrange("p (h b) -> p h b", h=2),
                        in0=cval[:, :].rearrange("p (h q b) -> p h q b",
                                                 h=2, q=3)[:, :, j, :],
                        in1=cwb[:].rearrange("p (h b) -> p h b", h=2),
                        op=mybir.AluOpType.mult)
                    cscs.append(csc)
                cov = wk.tile([128, 1024], bf16, name="cov", bufs=2)
                nc.vector.tensor_tensor(out=cov[:], in0=cscs[0][:],
                                        in1=cscs[1][:],
                                        op=mybir.AluOpType.add)
                for h in range(2):
                    nc.tensor.matmul(hp2[:, 512 * h:512 * (h + 1)],
                                     big[f"cw1b{i}"][:],
                                     cov[:, 512 * h:512 * (h + 1)],
                                     start=False, stop=True)
                for h in range(2):
                    it = 2 * ip + h
                    sl = slice(it * NT, (it + 1) * NT)
                    h_ = wk.tile([128, 512], bf16, name="h", bufs=3)
                    nc.scalar.activation(h_[:],
                                         hp2[:, 512 * h:512 * (h + 1)], LR,
                                         bias=bcol[f"cb1{i}"], alpha=0.01)
                    qp = pp.tile([2, 512], f32, name="qp", tag="T3")
                    nc.tensor.matmul(qp[:], cw2[i][:], h_[:], start=True,
                                     stop=True)
                    qs = wk.tile([2, 512], f32, name="qs", bufs=3)
                    nc.scalar.activation(qs[:], qp[:],
                                         mybir.ActivationFunctionType.Identity,
                                         bias=cb2t[:, i:i + 1])
                    nc.sync.dma_start(outd[2 * i:2 * i + 2, sl], qs[:])

    nc.compile()
    return nc


def _get_nc():
    if "nc" not in _NC_CACHE:
        _NC_CACHE["nc"] = _build_nc()
    return _NC_CACHE["nc"]


def kernel(s, a, en_W, en_b, oa_W, oa_b, goal_W, goal_b, akey_W, asel_W,
           aval_W, aval_b, merge_W, merge_b, senc_W, senc_b, ckey_W,
           csel_W, cval_W, cval_b, cW1, cb1, cW2, cb2):
    inp = dict(s=s, a=a, en_W=en_W, en_b=en_b, oa_W=oa_W, oa_b=oa_b,
               goal_W=goal_W, goal_b=goal_b, akey_W=akey_W, asel_W=asel_W,
               aval_W=aval_W, aval_b=aval_b, merge_W=merge_W, merge_b=merge_b,
               senc_W=senc_W, senc_b=senc_b, ckey_W=ckey_W, csel_W=csel_W,
               cval_W=cval_W, cval_b=cval_b, cW1=cW1, cb1=cb1, cW2=cW2,
               cb2=cb2)
    inp = {k: np.asarray(v, np.float32) for k, v in inp.items()}
    s_, a_ = inp["s"], inp["a"]

    l1w = _prep_l1w(inp)
    bigw = _prep_bigw(inp)
    cw2 = _b16(np.concatenate([inp["cW2"][n] for n in range(NA)], 0))
    biasc = _prep_bias(inp)
    cb2c = inp["cb2"].T.copy()

    in_maps = []
    for c in range(NCORES):
        ent = _prep_ent_blocks(s_, a_, c * BS, (c + 1) * BS)
        in_maps.append({"entd": ent, "l1wd": l1w, "bigwd": bigw,
                        "cw2d": cw2, "biasd": biasc, "cb2d": cb2c})

    nc = _get_nc()
    trace = os.environ.get("BASS_KERNEL_TRACE") == "1"
    res = run_bass_kernel_spmd(nc, in_maps, core_ids=list(range(NCORES)),
                               trace=trace)
    if trace:
        kernel.last_exec_time_ns = res.exec_time_ns
        kernel.last_results = res

    qfull = np.concatenate([res.results[c]["outd"] for c in range(NCORES)], 1)
    return np.ascontiguousarray(
        np.transpose(qfull.reshape(NA, 2, B), (0, 2, 1))).astype(np.float32)

